# revision 20
# baseline (speedup 1.0000x reference)
"""CP tensor reconstruction kernel for Trainium2 (8 NeuronCores).

Computes full[i0, i2, i1] = sum_r f0[i0,r] * f2[i2,r] * f1[i1,r],
returned flattened, for N0=512, N1=512, N2=256, R=32 (fp32).

Sharding: the output (512, 256, 512) is split into a 4x2 grid —
4 blocks of 128 i0-rows x 2 halves of 128 i2-values. Each of the 8
cores computes one (128, 128*512) slab. This gives full 128-partition
DMA stores (all 16 SBUF ports) and full M=128 matmuls.

Per core: for each i2, out_slice(128, 512) = f0_blk @ diag(f2[i2]) @ f1.T,
i.e. a (128x32)@(32x512) matmul where the rhs b[r, i1] = f1[i1,r]*f2[i2,r]
is built on the DVE with one per-partition-scalar multiply for 4 i2 at a
time (f1.T replicated on 4 partition groups). The K=32 matmuls are packed
4-way onto the PE array via tile_position row groups.
"""

import numpy as np

import concourse.bass as bass
import concourse.bacc as bacc
import concourse.mybir as mybir
from concourse.tile import TileContext
from concourse.bass_utils import run_bass_kernel_spmd

N0, N1, N2, R = 512, 512, 256, 32
NCORES = 8
I0_BLOCKS = 4  # i0 split
I2_BLOCKS = 2  # i2 split
I0_BLK = N0 // I0_BLOCKS  # 128
I2_BLK = N2 // I2_BLOCKS  # 128
OUT_COLS = I2_BLK * N1  # 65536 per-core slab columns

F32 = mybir.dt.float32

# i2-batches of 4 handled per DVE build op
NBATCH = I2_BLK // 4  # 32
# i2-values per staging tile / output DMA
STAGE_I2 = 4
STAGE_N = STAGE_I2 * N1  # 2048 (1 MiB fp32)
NSTAGE = I2_BLK // STAGE_I2  # 32


# fused constant input layout: [w (128) | f1t (512) | sc (32)] columns
CONST_COLS = I0_BLK + N1 + NBATCH  # 672
W_OFF = 0
F1_OFF = I0_BLK
SC_OFF = I0_BLK + N1


def _build_nc() -> bass.Bass:
    nc = bacc.Bacc("TRN2", target_bir_lowering=False)

    const_d = nc.dram_tensor("consts", [128, CONST_COLS], F32, kind="ExternalInput")
    out_d = nc.dram_tensor("out", [I0_BLK, OUT_COLS], F32, kind="ExternalOutput")

    with TileContext(nc) as tc:
        with (
            tc.tile_pool(name="const", bufs=1) as cpool,
            tc.tile_pool(name="bpool", bufs=8) as bpool,
            tc.tile_pool(name="psum", bufs=8, space="PSUM") as ppool,
            tc.tile_pool(name="stage", bufs=12) as spool,
        ):
            consts = cpool.tile([128, CONST_COLS], F32)
            nc.sync.dma_start(out=consts[:], in_=const_d[:])
            w = consts[:, W_OFF : W_OFF + I0_BLK]
            f1t = consts[:, F1_OFF : F1_OFF + N1]
            sct = consts[:, SC_OFF : SC_OFF + NBATCH]

            # Stage sizes in units of (matmul+copy) chunks of 512 columns.
            # First stages are smaller so the output DMA stream starts
            # sooner; steady-state stages are 4 chunks = 1 MiB.
            stage_sizes = [2, 2] + [4] * 31
            assert sum(stage_sizes) == 4 * NBATCH
            dma_engines = [nc.sync, nc.scalar]

            # flat generator over (batch t, rowgroup q) with build emission
            def chunks():
                for t in range(NBATCH):
                    b = bpool.tile([128, N1], F32, tag="b", name=f"b{t}")
                    nc.vector.tensor_scalar_mul(
                        out=b[:], in0=f1t, scalar1=sct[:, t : t + 1]
                    )
                    for q in range(4):
                        yield b, q

            gen = chunks()
            col_base = 0
            q_i = 0
            for s, size in enumerate(stage_sizes):
                stage = spool.tile([128, 4 * N1], F32, tag="stage", name=f"st{s}")
                for j in range(size):
                    b, q = next(gen)
                    ps = ppool.tile([128, N1], F32, tag="ps", name=f"ps{s}_{j}")
                    nc.tensor.matmul(
                        ps[:],
                        w[32 * q : 32 * q + 32, :],
                        b[32 * q : 32 * q + 32, :],
                        tile_position=(32 * q, 0),
                    )
                    col = j * N1
                    if q_i % 2 == 0:
                        nc.vector.tensor_copy(out=stage[:, col : col + N1], in_=ps[:])
                    else:
                        nc.scalar.copy(out=stage[:, col : col + N1], in_=ps[:])
                    q_i += 1
                ncols = size * N1
                dma_engines[s % 2].dma_start(
                    out=out_d[:, col_base : col_base + ncols], in_=stage[:, 0:ncols]
                )
                col_base += ncols
    nc.finalize()
    return nc


_NC = None


def _get_nc():
    global _NC
    if _NC is None:
        _NC = _build_nc()
    return _NC


def _make_consts(f0, f1, f2, c):
    i0b = c % I0_BLOCKS
    i2b = c // I0_BLOCKS
    f0_blk = f0[i0b * I0_BLK : (i0b + 1) * I0_BLK]  # (128, 32)
    w = np.tile(f0_blk.T, (4, 1))  # (128, 128)
    f1t = np.tile(f1.T, (4, 1))  # (128, 512)
    f2_blk = f2[i2b * I2_BLK : (i2b + 1) * I2_BLK]  # (128, 32)
    # sc[32q + r, t] = f2_blk[4t + q, r]
    sc = f2_blk.reshape(NBATCH, 4, R).transpose(1, 2, 0).reshape(128, NBATCH)
    return np.ascontiguousarray(
        np.concatenate([w, f1t, sc], axis=1), dtype=np.float32
    )


def kernel(f0, f1, f2):
    f0 = np.ascontiguousarray(np.asarray(f0), dtype=np.float32)
    f1 = np.ascontiguousarray(np.asarray(f1), dtype=np.float32)
    f2 = np.ascontiguousarray(np.asarray(f2), dtype=np.float32)
    assert f0.shape == (N0, R) and f1.shape == (N1, R) and f2.shape == (N2, R)

    nc = _get_nc()

    in_maps = [{"consts": _make_consts(f0, f1, f2, c)} for c in range(NCORES)]

    results = run_bass_kernel_spmd(nc, in_maps, core_ids=list(range(NCORES))).results

    full = np.empty((I0_BLOCKS, I0_BLK, I2_BLOCKS, I2_BLK * N1), dtype=np.float32)
    for c in range(NCORES):
        i0b = c % I0_BLOCKS
        i2b = c // I0_BLOCKS
        full[i0b, :, i2b, :] = results[c]["out"]
    return full.reshape(-1)


# revision 22
# speedup vs baseline: 1.0038x; 1.0038x over previous
"""CP tensor reconstruction kernel for Trainium2 (8 NeuronCores).

Computes full[i0, i2, i1] = sum_r f0[i0,r] * f2[i2,r] * f1[i1,r],
returned flattened, for N0=512, N1=512, N2=256, R=32 (fp32).

Sharding: the output (512, 256, 512) is split into a 4x2 grid —
4 blocks of 128 i0-rows x 2 halves of 128 i2-values. Each of the 8
cores computes one (128, 128*512) slab. This gives full 128-partition
DMA stores (all 16 SBUF ports) and full M=128 matmuls.

Per core: for each i2, out_slice(128, 512) = f0_blk @ diag(f2[i2]) @ f1.T,
i.e. a (128x32)@(32x512) matmul where the rhs b[r, i1] = f1[i1,r]*f2[i2,r]
is built on the DVE with one per-partition-scalar multiply for 4 i2 at a
time (f1.T replicated on 4 partition groups). The K=32 matmuls are packed
4-way onto the PE array via tile_position row groups.
"""

import numpy as np

import concourse.bass as bass
import concourse.bacc as bacc
import concourse.mybir as mybir
from concourse.tile import TileContext
from concourse.bass_utils import run_bass_kernel_spmd

N0, N1, N2, R = 512, 512, 256, 32
NCORES = 8
I0_BLOCKS = 4  # i0 split
I2_BLOCKS = 2  # i2 split
I0_BLK = N0 // I0_BLOCKS  # 128
I2_BLK = N2 // I2_BLOCKS  # 128
OUT_COLS = I2_BLK * N1  # 65536 per-core slab columns

F32 = mybir.dt.float32

# i2-batches of 4 handled per DVE build op
NBATCH = I2_BLK // 4  # 32
# i2-values per staging tile / output DMA
STAGE_I2 = 4
STAGE_N = STAGE_I2 * N1  # 2048 (1 MiB fp32)
NSTAGE = I2_BLK // STAGE_I2  # 32


# fused constant input layout: [w (128) | f1t (512) | sc (32)] columns
CONST_COLS = I0_BLK + N1 + NBATCH  # 672
W_OFF = 0
F1_OFF = I0_BLK
SC_OFF = I0_BLK + N1


def _build_nc() -> bass.Bass:
    nc = bacc.Bacc("TRN2", target_bir_lowering=False)

    const_d = nc.dram_tensor("consts", [128, CONST_COLS], F32, kind="ExternalInput")
    out_d = nc.dram_tensor("out", [I0_BLK, OUT_COLS], F32, kind="ExternalOutput")

    with TileContext(nc) as tc:
        with (
            tc.tile_pool(name="const", bufs=1) as cpool,
            tc.tile_pool(name="bpool", bufs=8) as bpool,
            tc.tile_pool(name="psum", bufs=8, space="PSUM") as ppool,
            tc.tile_pool(name="stage", bufs=12) as spool,
        ):
            consts = cpool.tile([128, CONST_COLS], F32)
            nc.sync.dma_start(out=consts[:], in_=const_d[:])
            w = consts[:, W_OFF : W_OFF + I0_BLK]
            f1t = consts[:, F1_OFF : F1_OFF + N1]
            sct = consts[:, SC_OFF : SC_OFF + NBATCH]

            # Stage sizes in units of (matmul+copy) chunks of 512 columns.
            # First stages are smaller so the output DMA stream starts
            # sooner; steady-state stages are 4 chunks = 1 MiB.
            stage_sizes = [2, 2] + [4] * 31
            assert sum(stage_sizes) == 4 * NBATCH

            # flat generator over (batch t, rowgroup q) with build emission
            def chunks():
                for t in range(NBATCH):
                    b = bpool.tile([128, N1], F32, tag="b", name=f"b{t}")
                    nc.vector.tensor_scalar_mul(
                        out=b[:], in0=f1t, scalar1=sct[:, t : t + 1]
                    )
                    for q in range(4):
                        yield b, q

            gen = chunks()
            col_base = 0
            q_i = 0
            for s, size in enumerate(stage_sizes):
                stage = spool.tile([128, 4 * N1], F32, tag="stage", name=f"st{s}")
                for j in range(size):
                    b, q = next(gen)
                    ps = ppool.tile([128, N1], F32, tag="ps", name=f"ps{s}_{j}")
                    nc.tensor.matmul(
                        ps[:],
                        w[32 * q : 32 * q + 32, :],
                        b[32 * q : 32 * q + 32, :],
                        tile_position=(32 * q, 0),
                    )
                    col = j * N1
                    if q_i % 2 == 0:
                        nc.vector.tensor_copy(out=stage[:, col : col + N1], in_=ps[:])
                    else:
                        nc.scalar.copy(out=stage[:, col : col + N1], in_=ps[:])
                    q_i += 1
                ncols = size * N1
                nc.sync.dma_start(
                    out=out_d[:, col_base : col_base + ncols], in_=stage[:, 0:ncols]
                )
                col_base += ncols
    nc.finalize()
    return nc


_NC = None


def _get_nc():
    global _NC
    if _NC is None:
        _NC = _build_nc()
    return _NC


def _make_consts(f0, f1, f2, c):
    i0b = c % I0_BLOCKS
    i2b = c // I0_BLOCKS
    f0_blk = f0[i0b * I0_BLK : (i0b + 1) * I0_BLK]  # (128, 32)
    w = np.tile(f0_blk.T, (4, 1))  # (128, 128)
    f1t = np.tile(f1.T, (4, 1))  # (128, 512)
    f2_blk = f2[i2b * I2_BLK : (i2b + 1) * I2_BLK]  # (128, 32)
    # sc[32q + r, t] = f2_blk[4t + q, r]
    sc = f2_blk.reshape(NBATCH, 4, R).transpose(1, 2, 0).reshape(128, NBATCH)
    return np.ascontiguousarray(
        np.concatenate([w, f1t, sc], axis=1), dtype=np.float32
    )


def kernel(f0, f1, f2):
    f0 = np.ascontiguousarray(np.asarray(f0), dtype=np.float32)
    f1 = np.ascontiguousarray(np.asarray(f1), dtype=np.float32)
    f2 = np.ascontiguousarray(np.asarray(f2), dtype=np.float32)
    assert f0.shape == (N0, R) and f1.shape == (N1, R) and f2.shape == (N2, R)

    nc = _get_nc()

    in_maps = [{"consts": _make_consts(f0, f1, f2, c)} for c in range(NCORES)]

    results = run_bass_kernel_spmd(nc, in_maps, core_ids=list(range(NCORES))).results

    full = np.empty((I0_BLOCKS, I0_BLK, I2_BLOCKS, I2_BLK * N1), dtype=np.float32)
    for c in range(NCORES):
        i0b = c % I0_BLOCKS
        i2b = c // I0_BLOCKS
        full[i0b, :, i2b, :] = results[c]["out"]
    return full.reshape(-1)
